# revision 16
# baseline (speedup 1.0000x reference)
"""Trainium2 Bass kernel for nn_CausalFullAttention_13735305413109.

Causal attention with a data-dependent cumprod decay gate and no softmax.
Because there is no softmax, the masked quadratic attention is algebraically
a chunked linear attention:
    out_i = q'_i @ State_{blk(i)} + sum_{j<=i, same blk} (q'_i.k'_j) v_j
    State_t = sum_{j < t*BLK} k'_j (x) v_j
with q' = q*SCALE*a_cum, k' = k/max(a_cum,1e-8), computed per (batch, head).

Sharding: head-parallel across 8 cores (head h -> core h, both batches local),
no cross-device communication; each core emits its partial output projection
out_h @ w_out[h*64:(h+1)*64, :] over all 4096 token rows, and the host sums
the 8 partials (+ b_out) as the unshard step.
"""
import numpy as np
from contextlib import ExitStack

import concourse.bass as bass
import concourse.bacc as bacc
import concourse.mybir as mybir
import concourse.tile as tile
from concourse.bass_utils import run_bass_kernel_spmd

F32 = mybir.dt.float32
AF = mybir.ActivationFunctionType
ALU = mybir.AluOpType

B = 2
N = 2048
DIM = 512
HEADS = 8
DH = 64
NTOK = B * N            # 4096 token rows
BLK = 128               # token block
NBLK = N // BLK         # 16 blocks per batch
PANEL = 512             # projection panel (moving free dim)
NPAN = NTOK // PANEL    # 8 panels
NCHUNK = DIM // 128     # 4 contraction chunks
SCALE = DH ** -0.5
LOG_SQRT_DIM = float(np.log(np.sqrt(DIM)))
EPS_INV = 1e-8


def build_nc(with_qkv_bias: bool):
    nc = bacc.Bacc()
    xT_d = nc.dram_tensor("xT", [DIM, NTOK], F32, kind="ExternalInput")
    wqk_d = nc.dram_tensor("wqk", [128, NCHUNK, 128], F32, kind="ExternalInput")
    wvz_d = nc.dram_tensor("wvz", [128, NCHUNK, 128], F32, kind="ExternalInput")
    wout_d = nc.dram_tensor("wout", [DH, DIM], F32, kind="ExternalInput")
    ba_d = nc.dram_tensor("ba", [DH, 1], F32, kind="ExternalInput")
    nba_d = nc.dram_tensor("nba", [DH, 1], F32, kind="ExternalInput")
    ident_d = nc.dram_tensor("ident", [128, 128], F32, kind="ExternalInput")
    mask_d = nc.dram_tensor("mask", [128, 128], F32, kind="ExternalInput")
    if with_qkv_bias:
        bqk_d = nc.dram_tensor("bqk", [128, 1], F32, kind="ExternalInput")
        bv_d = nc.dram_tensor("bv", [DH, 1], F32, kind="ExternalInput")
    y_d = nc.dram_tensor("ypart", [NTOK, DIM], F32, kind="ExternalOutput")

    with tile.TileContext(nc) as tc, ExitStack() as ctx:
        # ---- persistent sbuf ----
        per = ctx.enter_context(tc.tile_pool(name="persist", bufs=1))
        wqk_sb = per.tile([128, NCHUNK, 128], F32, tag="wqk")
        wvz_sb = per.tile([128, NCHUNK, 128], F32, tag="wvz")
        wout_sb = per.tile([DH, DIM], F32, tag="wout")
        ident_sb = per.tile([128, 128], F32, tag="ident")
        mask_sb = per.tile([128, 128], F32, tag="mask")
        ba_sb = per.tile([128, 1], F32, tag="ba")
        nba_sb = per.tile([128, 1], F32, tag="nba")
        ones_sb = per.tile([128, 128], F32, tag="ones")
        sRep = per.tile([128, NTOK], F32, tag="sRep")
        qk_sb = per.tile([128, NTOK], F32, tag="qk")      # rows 0:64 q'T, 64:128 k'T
        v_sb = per.tile([128, NTOK], F32, tag="v")        # rows 0:64 v'; 64:128 zs then ainv
        a_sb = per.tile([128, NTOK], F32, tag="a")        # rows 64:128 sigmoid(z)
        acum_sb = per.tile([128, NTOK], F32, tag="acum")  # rows 64:128 cumprod; 0:64 dup
        einv_sb = per.tile([128, NTOK], F32, tag="einv")  # rows 64:128 1+exp(-z)
        qdup = per.tile([128, NTOK], F32, tag="qdup")     # rows 64:128 q' copy

        nc.sync.dma_start(wqk_sb[:], wqk_d[:])
        nc.sync.dma_start(wvz_sb[:], wvz_d[:])
        nc.sync.dma_start(wout_sb[:], wout_d[:])
        nc.sync.dma_start(ident_sb[:], ident_d[:])
        nc.sync.dma_start(mask_sb[:], mask_d[:])
        nc.sync.dma_start(ba_sb[64:128, :], ba_d[:])
        nc.sync.dma_start(nba_sb[64:128, :], nba_d[:])
        if with_qkv_bias:
            bqk_sb = per.tile([128, 1], F32, tag="bqk")
            bv_sb = per.tile([128, 1], F32, tag="bv")
            nc.sync.dma_start(bqk_sb[:], bqk_d[:])
            nc.sync.dma_start(bv_sb[0:64, :], bv_d[:])
        nc.gpsimd.memset(ones_sb[:], 1.0)
        lsd_sb = per.tile([128, 1], F32, tag="lsd")
        nc.gpsimd.memset(lsd_sb[:], LOG_SQRT_DIM)

        # ---- phase A: load x, sumsq->s, projections ----
        with (
            tc.tile_pool(name="xt", bufs=2) as xtp,
            tc.tile_pool(name="x2", bufs=4) as x2p,
            tc.tile_pool(name="lns", bufs=2) as lnp,
            tc.tile_pool(name="ss_ps", bufs=2, space="PSUM") as ssp,
            tc.tile_pool(name="vz_ps", bufs=2, space="PSUM") as vzp,
            tc.tile_pool(name="qk_ps", bufs=2, space="PSUM") as qkp,
        ):
            for p in range(NPAN):
                cols = bass.ts(p, PANEL)
                xt = []
                for c in range(NCHUNK):
                    xc = xtp.tile([128, PANEL], F32, tag=f"xt{c}")
                    nc.sync.dma_start(xc[:], xT_d[128 * c:128 * (c + 1), cols])
                    xt.append(xc)
                # sum of squares -> replicated on all partitions via all-ones lhsT
                ss_ps = ssp.tile([128, PANEL], F32)
                for c in range(NCHUNK):
                    x2 = x2p.tile([128, PANEL], F32)
                    if c < 2:
                        nc.scalar.square(x2[:], xt[c][:])
                    elif c == 2:
                        nc.vector.tensor_mul(x2[:], xt[c][:], xt[c][:])
                    else:
                        nc.gpsimd.tensor_mul(x2[:], xt[c][:], xt[c][:])
                    nc.tensor.matmul(ss_ps[:], ones_sb[:], x2[:],
                                     start=(c == 0), stop=(c == NCHUNK - 1))
                # s = exp(-0.5*ln(ss) + ln(sqrt(DIM)))  (= sqrt(DIM)/||x_t||)
                lnt = lnp.tile([128, PANEL], F32)
                nc.scalar.activation(lnt[:], ss_ps[:], AF.Ln)
                nc.scalar.activation(sRep[:, cols], lnt[:], AF.Exp,
                                     bias=lsd_sb[:], scale=-0.5)
                # v|z projection, scaled by s at psum->sbuf
                vz_ps = vzp.tile([128, PANEL], F32)
                for c in range(NCHUNK):
                    nc.tensor.matmul(vz_ps[:], wvz_sb[:, c, :], xt[c][:],
                                     start=(c == 0), stop=(c == NCHUNK - 1))
                nc.vector.tensor_mul(v_sb[:, cols], vz_ps[:], sRep[:, cols])
                if with_qkv_bias:
                    nc.vector.tensor_scalar_add(v_sb[0:64, cols], v_sb[0:64, cols],
                                                bv_sb[0:64, :])
                # q|k projection, scaled by s at psum->sbuf
                qk_ps = qkp.tile([128, PANEL], F32)
                for c in range(NCHUNK):
                    nc.tensor.matmul(qk_ps[:], wqk_sb[:, c, :], xt[c][:],
                                     start=(c == 0), stop=(c == NCHUNK - 1))
                nc.vector.tensor_mul(qk_sb[:, cols], qk_ps[:], sRep[:, cols])
                if with_qkv_bias:
                    nc.vector.tensor_scalar_add(qk_sb[:, cols], qk_sb[:, cols],
                                                bqk_sb[:])

        # ---- phase B: decay gate per batch ----
        for b in range(B):
            C = slice(b * N, (b + 1) * N)
            zs = v_sb[64:128, C]
            nc.scalar.activation(a_sb[64:128, C], zs, AF.Sigmoid,
                                 bias=ba_sb[64:128, :])
            nc.vector.tensor_tensor_scan(acum_sb[64:128, C], a_sb[64:128, C],
                                         a_sb[64:128, C], 1.0, ALU.mult, ALU.bypass)
            nc.scalar.activation(einv_sb[64:128, C], zs, AF.Exp,
                                 bias=nba_sb[64:128, :], scale=-1.0)
            nc.gpsimd.tensor_scalar_add(einv_sb[64:128, C], einv_sb[64:128, C], 1.0)
            # ainv = min(cumprod(1+exp(-z)), 1e8)  == 1/max(cumprod(a), 1e-8)
            nc.vector.tensor_tensor_scan(v_sb[64:128, C], einv_sb[64:128, C],
                                         einv_sb[64:128, C], 1.0, ALU.mult, ALU.bypass)
            nc.gpsimd.tensor_scalar_min(v_sb[64:128, C], v_sb[64:128, C], 1.0 / EPS_INV)
            # A_q dup to rows 0:64, then scale q' and k' in place
            nc.sync.dma_start(acum_sb[0:64, C], acum_sb[64:128, C])
            nc.vector.tensor_mul(qk_sb[0:64, C], qk_sb[0:64, C], acum_sb[0:64, C])
            nc.vector.tensor_mul(qk_sb[64:128, C], qk_sb[64:128, C], v_sb[64:128, C])
            nc.sync.dma_start(qdup[64:128, C], qk_sb[0:64, C])

        # ---- phase C: chunked attention + output projection per batch ----
        with (
            tc.tile_pool(name="tm", bufs=4) as tmp,
            tc.tile_pool(name="ssb", bufs=2) as ssbp,
            tc.tile_pool(name="osb", bufs=2) as osbp,
            tc.tile_pool(name="stsb", bufs=2) as stsbp,
            tc.tile_pool(name="ysb", bufs=3) as ysbp,
            tc.tile_pool(name="tr_ps", bufs=2, space="PSUM") as trp,
            tc.tile_pool(name="s_ps", bufs=2, space="PSUM") as sp,
            tc.tile_pool(name="o_ps", bufs=1, space="PSUM") as op,
            tc.tile_pool(name="st_ps", bufs=1, space="PSUM") as stp,
            tc.tile_pool(name="y_ps", bufs=2, space="PSUM") as yp,
        ):
            for b in range(B):
                state_sb = stsbp.tile([64, 64], F32, tag="state")
                for t in range(NBLK):
                    cols = bass.ts(b * NBLK + t, BLK)
                    # token-major v and k' via PE transpose
                    vt_ps = trp.tile([128, DH], F32, tag="tr")
                    nc.tensor.transpose(vt_ps[:], v_sb[0:64, cols], ident_sb[0:64, 0:64])
                    vtm = tmp.tile([128, DH], F32, tag="vtm")
                    nc.scalar.copy(vtm[:], vt_ps[:])
                    kt_ps = trp.tile([128, DH], F32, tag="tr")
                    nc.tensor.transpose(kt_ps[:], qk_sb[64:128, cols],
                                        ident_sb[64:128, 64:128])
                    ktm = tmp.tile([128, DH], F32, tag="ktm")
                    nc.scalar.copy(ktm[:], kt_ps[:])
                    # S^T = k' q'^T on this block, masked to kt<=qt
                    s_ps = sp.tile([128, BLK], F32)
                    nc.tensor.matmul(s_ps[:], qk_sb[64:128, cols], qdup[64:128, cols],
                                     start=True, stop=True)
                    ssb = ssbp.tile([128, BLK], F32)
                    nc.vector.tensor_mul(ssb[:], s_ps[:], mask_sb[:])
                    # O^T = State^T q'^T (inter) + V^T S^T (intra)
                    o_ps = op.tile([64, BLK], F32)
                    if t > 0:
                        nc.tensor.matmul(o_ps[:], state_sb[:], qk_sb[0:64, cols],
                                         start=True, stop=False)
                    nc.tensor.matmul(o_ps[:], vtm[:], ssb[:],
                                     start=(t == 0), stop=True)
                    # State += K'^T V, accumulated in SBUF (psum groups can't
                    # be read mid-accumulation). Not needed after last block.
                    if t < NBLK - 1:
                        st_ps = stp.tile([64, 64], F32)
                        nc.tensor.matmul(st_ps[:], ktm[:], vtm[:],
                                         start=True, stop=True)
                        if t == 0:
                            nc.vector.tensor_copy(state_sb[:], st_ps[:])
                        else:
                            nc.vector.tensor_add(state_sb[:], state_sb[:], st_ps[:])
                    osb = osbp.tile([64, BLK], F32)
                    nc.scalar.copy(osb[:], o_ps[:])
                    # y = O @ wout_h   [128 tok, 512]
                    y_ps = yp.tile([128, DIM], F32)
                    nc.tensor.matmul(y_ps[:], osb[:], wout_sb[:], start=True, stop=True)
                    ysb = ysbp.tile([128, DIM], F32)
                    if t % 2 == 0:
                        nc.vector.tensor_copy(ysb[:], y_ps[:])
                    else:
                        nc.scalar.copy(ysb[:], y_ps[:])
                    r0 = b * N + t * BLK
                    nc.sync.dma_start(y_d[r0:r0 + BLK, :], ysb[:])
    nc.finalize()
    return nc


_NC_CACHE = {}


def _get_nc(with_qkv_bias: bool):
    if with_qkv_bias not in _NC_CACHE:
        _NC_CACHE[with_qkv_bias] = build_nc(with_qkv_bias)
    return _NC_CACHE[with_qkv_bias]


def make_in_maps(x, gamma, w_qkv, b_qkv, w_a, b_a, w_out, b_out, with_qkv_bias):
    x = np.asarray(x, np.float32)
    gamma = np.asarray(gamma, np.float32)
    w_qkv = np.asarray(w_qkv, np.float32)
    b_qkv = np.asarray(b_qkv, np.float32)
    w_a = np.asarray(w_a, np.float32)
    b_a = np.asarray(b_a, np.float32)

    xT = np.ascontiguousarray(x.reshape(NTOK, DIM).T)
    wq = w_qkv[:, 0:DIM] * gamma[:, None] * SCALE
    wk = w_qkv[:, DIM:2 * DIM] * gamma[:, None]
    wv = w_qkv[:, 2 * DIM:3 * DIM] * gamma[:, None]
    wa = w_a * gamma[:, None]
    ident = np.eye(128, dtype=np.float32)
    mask = np.triu(np.ones((128, 128), np.float32))  # [kt, qt] keep kt<=qt

    in_maps = []
    for h in range(HEADS):
        sl = slice(h * DH, (h + 1) * DH)
        wqk = np.concatenate([wq[:, sl], wk[:, sl]], axis=1)   # [512, 128]
        wvz = np.concatenate([wv[:, sl], wa[:, sl]], axis=1)   # [512, 128]
        m = {
            "xT": xT,
            "wqk": np.ascontiguousarray(wqk.reshape(NCHUNK, 128, 128).transpose(1, 0, 2)),
            "wvz": np.ascontiguousarray(wvz.reshape(NCHUNK, 128, 128).transpose(1, 0, 2)),
            "wout": np.ascontiguousarray(np.asarray(w_out, np.float32)[sl, :]),
            "ba": np.ascontiguousarray(b_a[sl][:, None]),
            "nba": np.ascontiguousarray(-b_a[sl][:, None]),
            "ident": ident,
            "mask": mask,
        }
        if with_qkv_bias:
            bq = b_qkv[0:DIM][sl] * SCALE
            bk = b_qkv[DIM:2 * DIM][sl]
            bv = b_qkv[2 * DIM:3 * DIM][sl]
            m["bqk"] = np.ascontiguousarray(
                np.concatenate([bq, bk])[:, None].astype(np.float32))
            m["bv"] = np.ascontiguousarray(bv[:, None].astype(np.float32))
        in_maps.append(m)
    return in_maps


def kernel(x, gamma, w_qkv, b_qkv, w_a, b_a, w_out, b_out, _profile=None):
    with_qkv_bias = bool(np.any(np.asarray(b_qkv)))
    nc = _get_nc(with_qkv_bias)
    in_maps = make_in_maps(x, gamma, w_qkv, b_qkv, w_a, b_a, w_out, b_out,
                           with_qkv_bias)
    kwargs = dict(_profile) if _profile else {}
    res = run_bass_kernel_spmd(nc, in_maps, core_ids=list(range(HEADS)), **kwargs)
    if _profile is not None:
        _profile["result"] = res
    out = res.results[0]["ypart"].astype(np.float32).copy()
    for i in range(1, HEADS):
        out += res.results[i]["ypart"]
    out += np.asarray(b_out, np.float32)[None, :]
    return out.reshape(B, N, DIM)


# revision 19
# speedup vs baseline: 1.3204x; 1.3204x over previous
"""Trainium2 Bass kernel for nn_CausalFullAttention_13735305413109.

Causal attention with a data-dependent cumprod decay gate and no softmax.
Because there is no softmax, the masked quadratic attention is algebraically
a chunked linear attention:
    out_i = q'_i @ State_{blk(i)} + sum_{j<=i, same blk} (q'_i.k'_j) v_j
    State_t = sum_{j < t*BLK} k'_j (x) v_j
with q' = q*SCALE*a_cum, k' = k/max(a_cum,1e-8), computed per (batch, head).

Sharding: head-parallel across 8 cores (head h -> core h, both batches local),
no cross-device communication; each core emits its partial output projection
out_h @ w_out[h*64:(h+1)*64, :] over all 4096 token rows, and the host sums
the 8 partials (+ b_out) as the unshard step.
"""
import numpy as np
from contextlib import ExitStack

import concourse.bass as bass
import concourse.bacc as bacc
import concourse.mybir as mybir
import concourse.tile as tile
from concourse.bass_utils import run_bass_kernel_spmd

F32 = mybir.dt.float32
AF = mybir.ActivationFunctionType
ALU = mybir.AluOpType

B = 2
N = 2048
DIM = 512
HEADS = 8
DH = 64
NTOK = B * N            # 4096 token rows
BLK = 128               # token block
NBLK = N // BLK         # 16 blocks per batch
PANEL = 512             # projection panel (moving free dim)
NPAN = NTOK // PANEL    # 8 panels
NCHUNK = DIM // 128     # 4 contraction chunks
SCALE = DH ** -0.5
LOG_SQRT_DIM = float(np.log(np.sqrt(DIM)))
EPS_INV = 1e-8


def build_nc(with_qkv_bias: bool):
    nc = bacc.Bacc()
    xT_d = nc.dram_tensor("xT", [DIM, NTOK], F32, kind="ExternalInput")
    wqk_d = nc.dram_tensor("wqk", [128, NCHUNK, 128], F32, kind="ExternalInput")
    wvz_d = nc.dram_tensor("wvz", [128, NCHUNK, 128], F32, kind="ExternalInput")
    wout_d = nc.dram_tensor("wout", [DH, DIM], F32, kind="ExternalInput")
    ba_d = nc.dram_tensor("ba", [128, 1], F32, kind="ExternalInput")
    nba_d = nc.dram_tensor("nba", [128, 1], F32, kind="ExternalInput")
    ident_d = nc.dram_tensor("ident", [128, 128], F32, kind="ExternalInput")
    mask_d = nc.dram_tensor("mask", [128, 128], F32, kind="ExternalInput")
    if with_qkv_bias:
        bqk_d = nc.dram_tensor("bqk", [128, 1], F32, kind="ExternalInput")
        bv_d = nc.dram_tensor("bv", [DH, 1], F32, kind="ExternalInput")
    y_d = nc.dram_tensor("ypart", [NTOK, DIM], F32, kind="ExternalOutput")

    with tile.TileContext(nc) as tc, ExitStack() as ctx:
        # ---- persistent sbuf ----
        per = ctx.enter_context(tc.tile_pool(name="persist", bufs=1))
        wqk_sb = per.tile([128, NCHUNK, 128], F32, tag="wqk")
        wvz_sb = per.tile([128, NCHUNK, 128], F32, tag="wvz")
        wout_sb = per.tile([DH, DIM], F32, tag="wout")
        ident_sb = per.tile([128, 128], F32, tag="ident")
        mask_sb = per.tile([128, 128], F32, tag="mask")
        ba_sb = per.tile([128, 1], F32, tag="ba")
        nba_sb = per.tile([128, 1], F32, tag="nba")
        ones_sb = per.tile([128, 128], F32, tag="ones")
        sRep = per.tile([128, NTOK], F32, tag="sRep")
        qk_sb = per.tile([128, NTOK], F32, tag="qk")      # rows 0:64 q'T, 64:128 k'T
        v_sb = per.tile([128, NTOK], F32, tag="v")        # rows 0:64 v'; 64:128 zs then k'
        qdup = per.tile([128, NTOK], F32, tag="qdup")     # rows 64:128 q' copy
        # batch-stacked decay pipeline tiles: rows 0:64 = batch0, 64:128 = batch1
        zstk = per.tile([128, N], F32, tag="zstk")    # z*s; later ainv (scan2 out)
        astk = per.tile([128, N], F32, tag="astk")    # sigmoid; later acum_b1 shift
        estk = per.tile([128, N], F32, tag="estk")    # 1+exp(-z); later ainv_b0 shift
        acstk = per.tile([128, N], F32, tag="acstk")  # cumprod(a)

        nc.sync.dma_start(wqk_sb[:], wqk_d[:])
        nc.sync.dma_start(wvz_sb[:], wvz_d[:])
        nc.sync.dma_start(wout_sb[:], wout_d[:])
        nc.sync.dma_start(ident_sb[:], ident_d[:])
        nc.sync.dma_start(mask_sb[:], mask_d[:])
        nc.sync.dma_start(ba_sb[:], ba_d[:])
        nc.sync.dma_start(nba_sb[:], nba_d[:])
        if with_qkv_bias:
            bqk_sb = per.tile([128, 1], F32, tag="bqk")
            bv_sb = per.tile([128, 1], F32, tag="bv")
            nc.sync.dma_start(bqk_sb[:], bqk_d[:])
            nc.sync.dma_start(bv_sb[0:64, :], bv_d[:])
        nc.gpsimd.memset(ones_sb[:], 1.0)
        lsd_sb = per.tile([128, 1], F32, tag="lsd")
        nc.gpsimd.memset(lsd_sb[:], LOG_SQRT_DIM)

        # ---- phase A: load x, sumsq->s, projections ----
        with (
            tc.tile_pool(name="xt", bufs=2) as xtp,
            tc.tile_pool(name="x2", bufs=4) as x2p,
            tc.tile_pool(name="lns", bufs=2) as lnp,
            tc.tile_pool(name="ss_ps", bufs=2, space="PSUM") as ssp,
            tc.tile_pool(name="vz_ps", bufs=2, space="PSUM") as vzp,
            tc.tile_pool(name="qk_ps", bufs=2, space="PSUM") as qkp,
        ):
            for p in range(NPAN):
                cols = bass.ts(p, PANEL)
                xt = []
                for c in range(NCHUNK):
                    xc = xtp.tile([128, PANEL], F32, tag=f"xt{c}")
                    nc.sync.dma_start(xc[:], xT_d[128 * c:128 * (c + 1), cols])
                    xt.append(xc)
                # sum of squares -> replicated on all partitions via all-ones lhsT
                ss_ps = ssp.tile([128, PANEL], F32)
                for c in range(NCHUNK):
                    x2 = x2p.tile([128, PANEL], F32)
                    if c == 0:
                        nc.scalar.square(x2[:], xt[c][:])
                    elif c == 1:
                        nc.vector.tensor_mul(x2[:], xt[c][:], xt[c][:])
                    else:
                        nc.gpsimd.tensor_mul(x2[:], xt[c][:], xt[c][:])
                    nc.tensor.matmul(ss_ps[:], ones_sb[:], x2[:],
                                     start=(c == 0), stop=(c == NCHUNK - 1))
                # s = exp(-0.5*ln(ss) + ln(sqrt(DIM)))  (= sqrt(DIM)/||x_t||)
                lnt = lnp.tile([128, PANEL], F32)
                nc.scalar.activation(lnt[:], ss_ps[:], AF.Ln)
                nc.scalar.activation(sRep[:, cols], lnt[:], AF.Exp,
                                     bias=lsd_sb[:], scale=-0.5)
                # v|z projection, scaled by s at psum->sbuf
                vz_ps = vzp.tile([128, PANEL], F32)
                for c in range(NCHUNK):
                    nc.tensor.matmul(vz_ps[:], wvz_sb[:, c, :], xt[c][:],
                                     start=(c == 0), stop=(c == NCHUNK - 1))
                nc.vector.tensor_mul(v_sb[:, cols], vz_ps[:], sRep[:, cols])
                if with_qkv_bias:
                    nc.vector.tensor_scalar_add(v_sb[0:64, cols], v_sb[0:64, cols],
                                                bv_sb[0:64, :])
                # q|k projection, scaled by s at psum->sbuf
                qk_ps = qkp.tile([128, PANEL], F32)
                for c in range(NCHUNK):
                    nc.tensor.matmul(qk_ps[:], wqk_sb[:, c, :], xt[c][:],
                                     start=(c == 0), stop=(c == NCHUNK - 1))
                nc.vector.tensor_mul(qk_sb[:, cols], qk_ps[:], sRep[:, cols])
                if with_qkv_bias:
                    nc.vector.tensor_scalar_add(qk_sb[:, cols], qk_sb[:, cols],
                                                bqk_sb[:])

        # ---- phase B: decay gate, both batches stacked on the partition axis ----
        H0, H1, FB = slice(0, 64), slice(64, 128), slice(0, N)
        C0, C1 = slice(0, N), slice(N, 2 * N)
        # zstk rows 0:64 = zs(b0), rows 64:128 = zs(b1)
        nc.sync.dma_start(zstk[H0, FB], v_sb[H1, C0])
        nc.sync.dma_start(zstk[H1, FB], v_sb[H1, C1])
        nc.scalar.activation(astk[:], zstk[:], AF.Sigmoid, bias=ba_sb[:])
        nc.scalar.activation(estk[:], zstk[:], AF.Exp, bias=nba_sb[:], scale=-1.0)
        nc.vector.tensor_scalar_add(estk[:], estk[:], 1.0)
        nc.vector.tensor_tensor_scan(acstk[:], astk[:], astk[:], 1.0,
                                     ALU.mult, ALU.bypass)
        # ainv = min(cumprod(1+exp(-z)), 1e8) == 1/max(cumprod(a), 1e-8)
        nc.vector.tensor_tensor_scan(zstk[:], estk[:], estk[:], 1.0,
                                     ALU.mult, ALU.bypass)
        nc.vector.tensor_scalar_min(zstk[:], zstk[:], 1.0 / EPS_INV)
        # partition shifts so each consumer sees its operand on its own lanes
        nc.sync.dma_start(estk[H1, FB], zstk[H0, FB])    # ainv(b0) -> rows 64:
        nc.sync.dma_start(astk[H0, FB], acstk[H1, FB])   # acum(b1) -> rows 0:
        # q' = q * s * a_cum ; k' = k * s * ainv (k' written into v_sb rows 64:
        # so that one PE transpose per block yields both v'tm and k'tm)
        nc.vector.tensor_mul(qk_sb[H0, C0], qk_sb[H0, C0], acstk[H0, FB])
        nc.vector.tensor_mul(qk_sb[H0, C1], qk_sb[H0, C1], astk[H0, FB])
        nc.vector.tensor_mul(v_sb[H1, C0], qk_sb[H1, C0], estk[H1, FB])
        nc.vector.tensor_mul(v_sb[H1, C1], qk_sb[H1, C1], zstk[H1, FB])
        nc.sync.dma_start(qdup[H1, C0], qk_sb[H0, C0])
        nc.sync.dma_start(qdup[H1, C1], qk_sb[H0, C1])

        # ---- phase C: chunked attention + output projection, batches interleaved ----
        with (
            tc.tile_pool(name="vk", bufs=4) as vkp,
            tc.tile_pool(name="ssb", bufs=3) as ssbp,
            tc.tile_pool(name="osb", bufs=3) as osbp,
            tc.tile_pool(name="stsb", bufs=1) as stsbp,
            tc.tile_pool(name="ysb", bufs=3) as ysbp,
            tc.tile_pool(name="psA", bufs=3, space="PSUM") as psA,
            tc.tile_pool(name="psB", bufs=3, space="PSUM") as psB,
            tc.tile_pool(name="psY", bufs=2, space="PSUM") as psY,
        ):
            state_sb = [stsbp.tile([64, 64], F32, tag=f"state{b}",
                                   name=f"state_sb{b}") for b in range(B)]
            for t in range(NBLK):
                for b in range(B):
                    cols = bass.ts(b * NBLK + t, BLK)
                    # one transpose yields [v'tm | k'tm] (v_sb rows: 0:64 v', 64:128 k')
                    tr_ps = psA.tile([128, 128], F32, tag="blk")
                    nc.tensor.transpose(tr_ps[:], v_sb[:, cols], ident_sb[:])
                    vk = vkp.tile([128, 128], F32)
                    if (t + b) % 2 == 0:
                        nc.vector.tensor_copy(vk[:], tr_ps[:])
                    else:
                        nc.scalar.copy(vk[:], tr_ps[:])
                    # S^T = k' q'^T on this block, masked to kt<=qt
                    s_ps = psA.tile([128, BLK], F32, tag="blk")
                    nc.tensor.matmul(s_ps[:], v_sb[64:128, cols], qdup[64:128, cols],
                                     start=True, stop=True)
                    ssb = ssbp.tile([128, BLK], F32)
                    nc.vector.tensor_mul(ssb[:], s_ps[:], mask_sb[:])
                    # O^T = State^T q'^T (inter) + V^T S^T (intra)
                    o_ps = psB.tile([64, BLK], F32, tag="ob")
                    if t > 0:
                        nc.tensor.matmul(o_ps[:], state_sb[b][:], qk_sb[0:64, cols],
                                         start=True, stop=False)
                    nc.tensor.matmul(o_ps[:], vk[:, 0:64], ssb[:],
                                     start=(t == 0), stop=True)
                    # State += K'^T V, accumulated in SBUF
                    if t < NBLK - 1:
                        st_ps = psB.tile([64, 64], F32, tag="ob")
                        nc.tensor.matmul(st_ps[:], vk[:, 64:128], vk[:, 0:64],
                                         start=True, stop=True)
                        if t == 0:
                            nc.vector.tensor_copy(state_sb[b][:], st_ps[:])
                        else:
                            nc.vector.tensor_add(state_sb[b][:], state_sb[b][:],
                                                 st_ps[:])
                    osb = osbp.tile([64, BLK], F32)
                    nc.scalar.copy(osb[:], o_ps[:])
                    # y = O @ wout_h   [128 tok, 512]
                    y_ps = psY.tile([128, DIM], F32)
                    nc.tensor.matmul(y_ps[:], osb[:], wout_sb[:], start=True, stop=True)
                    ysb = ysbp.tile([128, DIM], F32)
                    if (t + b) % 2 == 0:
                        nc.vector.tensor_copy(ysb[:], y_ps[:])
                    else:
                        nc.scalar.copy(ysb[:], y_ps[:])
                    r0 = b * N + t * BLK
                    nc.sync.dma_start(y_d[r0:r0 + BLK, :], ysb[:])
    nc.finalize()
    return nc


_NC_CACHE = {}


def _get_nc(with_qkv_bias: bool):
    if with_qkv_bias not in _NC_CACHE:
        _NC_CACHE[with_qkv_bias] = build_nc(with_qkv_bias)
    return _NC_CACHE[with_qkv_bias]


def make_in_maps(x, gamma, w_qkv, b_qkv, w_a, b_a, w_out, b_out, with_qkv_bias):
    x = np.asarray(x, np.float32)
    gamma = np.asarray(gamma, np.float32)
    w_qkv = np.asarray(w_qkv, np.float32)
    b_qkv = np.asarray(b_qkv, np.float32)
    w_a = np.asarray(w_a, np.float32)
    b_a = np.asarray(b_a, np.float32)

    xT = np.ascontiguousarray(x.reshape(NTOK, DIM).T)
    wq = w_qkv[:, 0:DIM] * gamma[:, None] * SCALE
    wk = w_qkv[:, DIM:2 * DIM] * gamma[:, None]
    wv = w_qkv[:, 2 * DIM:3 * DIM] * gamma[:, None]
    wa = w_a * gamma[:, None]
    ident = np.eye(128, dtype=np.float32)
    mask = np.triu(np.ones((128, 128), np.float32))  # [kt, qt] keep kt<=qt

    in_maps = []
    for h in range(HEADS):
        sl = slice(h * DH, (h + 1) * DH)
        wqk = np.concatenate([wq[:, sl], wk[:, sl]], axis=1)   # [512, 128]
        wvz = np.concatenate([wv[:, sl], wa[:, sl]], axis=1)   # [512, 128]
        m = {
            "xT": xT,
            "wqk": np.ascontiguousarray(wqk.reshape(NCHUNK, 128, 128).transpose(1, 0, 2)),
            "wvz": np.ascontiguousarray(wvz.reshape(NCHUNK, 128, 128).transpose(1, 0, 2)),
            "wout": np.ascontiguousarray(np.asarray(w_out, np.float32)[sl, :]),
            "ba": np.ascontiguousarray(np.tile(b_a[sl], 2)[:, None]),
            "nba": np.ascontiguousarray(np.tile(-b_a[sl], 2)[:, None]),
            "ident": ident,
            "mask": mask,
        }
        if with_qkv_bias:
            bq = b_qkv[0:DIM][sl] * SCALE
            bk = b_qkv[DIM:2 * DIM][sl]
            bv = b_qkv[2 * DIM:3 * DIM][sl]
            m["bqk"] = np.ascontiguousarray(
                np.concatenate([bq, bk])[:, None].astype(np.float32))
            m["bv"] = np.ascontiguousarray(bv[:, None].astype(np.float32))
        in_maps.append(m)
    return in_maps


def kernel(x, gamma, w_qkv, b_qkv, w_a, b_a, w_out, b_out, _profile=None):
    with_qkv_bias = bool(np.any(np.asarray(b_qkv)))
    nc = _get_nc(with_qkv_bias)
    in_maps = make_in_maps(x, gamma, w_qkv, b_qkv, w_a, b_a, w_out, b_out,
                           with_qkv_bias)
    kwargs = dict(_profile) if _profile else {}
    res = run_bass_kernel_spmd(nc, in_maps, core_ids=list(range(HEADS)), **kwargs)
    if _profile is not None:
        _profile["result"] = res
    out = res.results[0]["ypart"].astype(np.float32).copy()
    for i in range(1, HEADS):
        out += res.results[i]["ypart"]
    out += np.asarray(b_out, np.float32)[None, :]
    return out.reshape(B, N, DIM)
